# revision 1
# baseline (speedup 1.0000x reference)
"""Trainium2 Bass kernel for nn_JointRelationModule (self-contained).

Math (per person p, all within one imgid group for the softmax):
    q = Wq x + bq ; k = Wk x + bk ; v = Wv x + bv          (1x1 conv over K=17)
    S_p = q_p k_p^T / 64                                   ([17,17] scores)
    attn = segment-softmax over the person dim (per imgid group, per (i,j))
    out = relu(attn_p @ v_p + x_p)

Key reformulation used on device: with G_p = x_p x_p^T (17x17 Gram),
    S_p = Wq G_p Wk^T / 64 (+ cheap rank-1 bias terms)
    attn_p @ v_p = (attn_p @ Wv) @ x_p (+ (attn_p @ bv) broadcast)
so the only O(p*K*hw) device work is: transpose x (PE), Gram (PE), and the
final (attn Wv) @ x matmul (PE, float32r) + residual/relu (DVE/ACT).

Sharding: data-parallel over persons, split at imgid group boundaries
(8 cores), weights replicated. Segment softmax runs fully on-device via
indicator-matrix matmuls (persons on partitions); the indicator is built on
the host from imgid (sharding metadata, not compute).
"""

import math
import sys

import numpy as np

K = 17
HW = 4096  # 64*64
P_TOTAL = 512
N_CORES = 8
NORM = 64.0
BD = 7          # persons per block-diagonal stack
BDK = BD * K    # 119
D_CH = 128      # transpose / gram chunk along hw dim
O_CH = 512      # output chunk along hw dim (one PSUM bank of f32)

_cache: dict = {}


def _ensure_path():
    try:
        import concourse.bass  # noqa: F401
    except ImportError:
        for p in ("/opt/trn_rl_repo", "/root/.axon_site/_ro/trn_rl_repo"):
            if p not in sys.path:
                sys.path.insert(0, p)
        import concourse.bass  # noqa: F401


def _build(P_pad: int, G_pad: int):
    """Builds + compiles the per-core SPMD Bass program."""
    _ensure_path()
    import concourse.bacc as bacc
    import concourse.mybir as mybir
    import concourse.tile as tile

    f32 = mybir.dt.float32
    bf16 = mybir.dt.bfloat16
    Exp = mybir.ActivationFunctionType.Exp
    Relu = mybir.ActivationFunctionType.Relu

    S = P_pad // BD
    assert P_pad % BD == 0 and P_pad <= 128 and G_pad <= 128
    n_dch = HW // D_CH   # 32
    n_och = HW // O_CH   # 8
    resident = S <= 10   # all of x stays in SBUF

    nc = bacc.Bacc(
        "TRN2",
        target_bir_lowering=False,
        debug=False,
        enable_asserts=False,
        num_devices=N_CORES,
    )

    x_d = nc.dram_tensor("x", [P_pad * K, HW], f32, kind="ExternalInput")
    wq_d = nc.dram_tensor("wq64t_bd", [BDK, BDK], f32, kind="ExternalInput")
    wk_d = nc.dram_tensor("wkt_bd", [BDK, BDK], f32, kind="ExternalInput")
    wv_d = nc.dram_tensor("wv_bd", [BDK, BDK], f32, kind="ExternalInput")
    i_d = nc.dram_tensor("i119", [BDK, BDK], f32, kind="ExternalInput")
    ind_d = nc.dram_tensor("ind", [P_pad, G_pad], f32, kind="ExternalInput")
    indt_d = nc.dram_tensor("indT", [G_pad, P_pad], f32, kind="ExternalInput")
    corr_d = nc.dram_tensor("corr", [P_pad, K * K], f32, kind="ExternalInput")
    bv_d = nc.dram_tensor("bv119", [BDK, 1], f32, kind="ExternalInput")
    y_d = nc.dram_tensor("y", [P_pad * K, HW], f32, kind="ExternalOutput")

    with tile.TileContext(nc) as tc:
        with (
            tc.tile_pool(name="xpool", bufs=1) as xpool,
            tc.tile_pool(name="cpool", bufs=1) as cpool,
            tc.tile_pool(name="wpool", bufs=2) as wpool,
            tc.tile_pool(name="fpool", bufs=1) as fpool,
            tc.tile_pool(name="opool", bufs=3) as opool,
            tc.tile_pool(name="pp", bufs=2, space="PSUM") as pp,
        ):
            # --- replicated constants ---
            wq_t = cpool.tile([BDK, BDK], f32, name="wq_t", tag="wq")
            wk_t = cpool.tile([BDK, BDK], f32, name="wk_t", tag="wk")
            wv_t = cpool.tile([BDK, BDK], f32, name="wv_t", tag="wv")
            id_t = cpool.tile([BDK, BDK], f32, name="id_t", tag="id")
            ind_t = cpool.tile([P_pad, G_pad], f32, name="ind_t", tag="ind")
            indt_t = cpool.tile([G_pad, P_pad], f32, name="indt_t", tag="indt")
            bv_t = cpool.tile([BDK, 1], f32, name="bv_t", tag="bv")
            nc.sync.dma_start(wq_t[:], wq_d.ap())
            nc.sync.dma_start(wk_t[:], wk_d.ap())
            nc.sync.dma_start(wv_t[:], wv_d.ap())
            nc.sync.dma_start(id_t[:], i_d.ap())
            nc.sync.dma_start(ind_t[:], ind_d.ap())
            nc.sync.dma_start(indt_t[:], indt_d.ap())
            nc.sync.dma_start(bv_t[:], bv_d.ap())

            e_flat = fpool.tile([P_pad, K * K], f32, name="e_flat", tag="e")
            corr_t = fpool.tile([P_pad, K * K], f32, name="corr_t", tag="corr")
            nc.sync.dma_start(corr_t[:], corr_d.ap())

            # --- phase A+B: per stack, gram -> scores^T -> extract ---
            x_tiles = []
            ncopy = 0
            for s in range(S):
                if resident:
                    xs = xpool.tile([BDK, HW], f32, name=f"xs{s}", tag=f"xs{s}")
                else:
                    xs = xpool.tile([BDK, HW], f32, name=f"xs{s}", tag="xs",
                                    bufs=3)
                # chunked load: spreads across DMA queues and lets the first
                # transposes start ~8x earlier than one monolithic 1.95MB DMA
                for lc in range(8):
                    lsl = slice(512 * lc, 512 * (lc + 1))
                    nc.sync.dma_start(
                        xs[:, lsl], x_d.ap()[BDK * s:BDK * (s + 1), lsl]
                    )
                x_tiles.append(xs)

                g_ps = pp.tile([BDK, BDK], f32, name=f"g{s}", tag="g", bufs=2)
                for dc in range(n_dch):
                    tp = pp.tile([D_CH, BDK], f32, name="tp", tag="tp", bufs=2)
                    nc.tensor.transpose(
                        tp[:], xs[:, D_CH * dc:D_CH * (dc + 1)], id_t[:]
                    )
                    xt_sb = wpool.tile([D_CH, BDK], f32, name="xt_sb", tag="xt")
                    # split PSUM->SBUF copies between DVE and ACT
                    if ncopy % 3 == 0:
                        nc.vector.tensor_copy(xt_sb[:], tp[:])
                    else:
                        nc.scalar.copy(xt_sb[:], tp[:])
                    ncopy += 1
                    nc.tensor.matmul(
                        g_ps[:], xt_sb[:], xt_sb[:],
                        start=(dc == 0), stop=(dc == n_dch - 1),
                    )

                # tiny chain: ST_stack = BD(Wk) @ (G @ BD(Wq^T/64))
                g_sb = wpool.tile([BDK, BDK], f32, name="g_sb", tag="g_sb")
                nc.vector.tensor_copy(g_sb[:], g_ps[:])
                m1_ps = pp.tile([BDK, BDK], f32, name="m1", tag="tiny", bufs=2)
                nc.tensor.matmul(m1_ps[:], g_sb[:], wq_t[:], start=True, stop=True)
                m1_sb = wpool.tile([BDK, BDK], f32, name="m1_sb", tag="m1_sb")
                nc.scalar.copy(m1_sb[:], m1_ps[:])
                st_ps = pp.tile([BDK, BDK], f32, name="st", tag="tiny", bufs=2)
                nc.tensor.matmul(st_ps[:], wk_t[:], m1_sb[:], start=True, stop=True)
                st_sb = wpool.tile([BDK, BDK], f32, name="st_sb", tag="st_sb")
                nc.vector.tensor_copy(st_sb[:], st_ps[:])
                for j in range(BD):
                    p = BD * s + j
                    nc.gpsimd.dma_start(
                        e_flat[p:p + 1, :],
                        st_sb[K * j:K * (j + 1), K * j:K * (j + 1)],
                    )

            # --- phase C: segment softmax over persons (on partitions) ---
            e_bias = fpool.tile([P_pad, K * K], f32, name="e_bias", tag="eb")
            nc.vector.tensor_add(e_bias[:], e_flat[:], corr_t[:])
            exp_flat = fpool.tile([P_pad, K * K], f32, name="exp_flat", tag="exp")
            nc.scalar.activation(exp_flat[:], e_bias[:], Exp)
            seg_ps = pp.tile([G_pad, K * K], f32, name="seg", tag="tiny", bufs=2)
            nc.tensor.matmul(seg_ps[:], ind_t[:], exp_flat[:], start=True, stop=True)
            seg_sb = fpool.tile([G_pad, K * K], f32, name="seg_sb", tag="seg")
            nc.vector.tensor_scalar_max(seg_sb[:], seg_ps[:], 1e-30)
            inv_sb = fpool.tile([G_pad, K * K], f32, name="inv_sb", tag="inv")
            nc.vector.reciprocal(inv_sb[:], seg_sb[:])
            invb_ps = pp.tile([P_pad, K * K], f32, name="invb", tag="tiny", bufs=2)
            nc.tensor.matmul(invb_ps[:], indt_t[:], inv_sb[:], start=True, stop=True)
            attn_flat = fpool.tile([P_pad, K * K], f32, name="attn_flat", tag="at")
            nc.vector.tensor_mul(attn_flat[:], exp_flat[:], invb_ps[:])

            # --- phase D: AT = BD(Wv^T attn^T); out = relu(AT.T @ x + x) ---
            for s in range(S):
                bdat = wpool.tile([BDK, BDK], f32, name="bdat", tag="bdat")
                nc.gpsimd.memset(bdat[:], 0.0)
                for j in range(BD):
                    p = BD * s + j
                    nc.gpsimd.dma_start(
                        bdat[K * j:K * (j + 1), K * j:K * (j + 1)],
                        attn_flat[p:p + 1, :],
                    )
                at_ps = pp.tile([BDK, BDK], f32, name="at", tag="tiny", bufs=2)
                nc.tensor.matmul(at_ps[:], wv_t[:], bdat[:], start=True, stop=True)
                at_sb = wpool.tile([BDK, BDK], bf16, name="at_sb", tag="at_sb")
                nc.scalar.copy(at_sb[:], at_ps[:])
                # attnv[17j+i] = sum_m attn^T[m,i] bv[m]  (v-bias broadcast term)
                av_ps = pp.tile([BDK, 1], f32, name="av", tag="tiny", bufs=2)
                nc.tensor.matmul(av_ps[:], bdat[:], bv_t[:], start=True, stop=True)
                av_sb = wpool.tile([BDK, 1], f32, name="av_sb", tag="av_sb")
                nc.vector.tensor_copy(av_sb[:], av_ps[:])

                for oc in range(n_och):
                    sl = slice(O_CH * oc, O_CH * (oc + 1))
                    if resident:
                        xr = x_tiles[s]
                        x_ap = xr[:, sl]
                    else:
                        xchunk = opool.tile([BDK, O_CH], f32, name="xchunk",
                                            tag="xc")
                        nc.sync.dma_start(
                            xchunk[:], x_d.ap()[BDK * s:BDK * (s + 1), sl]
                        )
                        x_ap = xchunk[:]
                    xbf = opool.tile([BDK, O_CH], bf16, name="xbf", tag="xbf")
                    nc.vector.tensor_copy(xbf[:], x_ap)
                    o_ps = pp.tile([BDK, O_CH], f32, name="o_ps", tag="ops", bufs=2)
                    nc.tensor.matmul(
                        o_ps[:], at_sb[:], xbf[:], start=True, stop=True,
                    )
                    sum_sb = opool.tile([BDK, O_CH], f32, name="sum_sb", tag="sum")
                    nc.vector.tensor_add(sum_sb[:], o_ps[:], x_ap)
                    res_sb = opool.tile([BDK, O_CH], f32, name="res_sb", tag="res")
                    nc.scalar.activation(res_sb[:], sum_sb[:], Relu,
                                         bias=av_sb[:, 0:1])
                    (nc.sync if oc % 2 == 0 else nc.gpsimd).dma_start(
                        y_d.ap()[BDK * s:BDK * (s + 1), sl], res_sb[:]
                    )

    nc.compile()
    return nc


def _get_compiled(P_pad: int, G_pad: int):
    key = (P_pad, G_pad)
    if key not in _cache:
        _cache[key] = _build(P_pad, G_pad)
    return _cache[key]


def _bd7(m: np.ndarray) -> np.ndarray:
    out = np.zeros((BDK, BDK), dtype=np.float32)
    for j in range(BD):
        out[K * j:K * (j + 1), K * j:K * (j + 1)] = m
    return out


def _plan(ids: np.ndarray):
    """Split persons into N_CORES contiguous chunks at imgid boundaries."""
    change = np.flatnonzero(np.diff(ids)) + 1
    allb = np.concatenate([[0], change, [P_TOTAL]]).astype(np.int64)
    bounds = [0]
    for ci in range(1, N_CORES):
        target = P_TOTAL * ci / N_CORES
        cand = allb[allb > bounds[-1]]
        if len(cand) == 0:
            bounds.append(bounds[-1])
        else:
            bounds.append(int(cand[np.argmin(np.abs(cand - target))]))
    bounds.append(P_TOTAL)
    sizes = np.diff(bounds)
    P_max = int(sizes.max())
    P_pad = max(BD, BD * math.ceil(P_max / BD))
    g_max = 0
    for ci in range(N_CORES):
        a, b = bounds[ci], bounds[ci + 1]
        g_max = max(g_max, len(np.unique(ids[a:b])))
    G_pad = max(4, 4 * math.ceil((g_max + 1) / 4))
    return bounds, P_pad, G_pad


def _prepare(inputs: dict):
    x = np.ascontiguousarray(
        np.asarray(inputs["kpt_feat"], dtype=np.float32).reshape(P_TOTAL, K, HW)
    )
    ids = np.asarray(inputs["imgid"]).astype(np.int64)
    Wq = np.asarray(inputs["Wq"], np.float32)
    Wk = np.asarray(inputs["Wk"], np.float32)
    Wv = np.asarray(inputs["Wv"], np.float32)
    bq = np.asarray(inputs["bq"], np.float32)
    bk = np.asarray(inputs["bk"], np.float32)
    bv = np.asarray(inputs["bv"], np.float32)

    bounds, P_pad, G_pad = _plan(ids)

    wq64t = _bd7((Wq.T / NORM).astype(np.float32))
    wkt = _bd7(Wk.T.astype(np.float32))
    wvb = _bd7(Wv.astype(np.float32))
    i119 = np.eye(BDK, dtype=np.float32)
    bv119 = np.tile(bv.reshape(K, 1), (BD, 1)).astype(np.float32)

    have_bias = bool(np.any(bq) or np.any(bk))
    if have_bias:
        xsum = x.sum(axis=2)                    # [P, K]
        qx = xsum @ Wq.T                        # [P, i]
        kx = xsum @ Wk.T                        # [P, m]
        corr_all = (
            bk[None, :, None] * qx[:, None, :]
            + bq[None, None, :] * kx[:, :, None]
            + HW * (bq[None, None, :] * bk[None, :, None])
        ) / NORM                                # [P, m, i]
        corr_all = corr_all.reshape(P_TOTAL, K * K).astype(np.float32)
    else:
        corr_all = np.zeros((P_TOTAL, K * K), dtype=np.float32)

    in_maps = []
    for ci in range(N_CORES):
        a, b = bounds[ci], bounds[ci + 1]
        pc = b - a
        xs = np.zeros((P_pad * K, HW), dtype=np.float32)
        if pc:
            xs[:pc * K] = x[a:b].reshape(pc * K, HW)
        corr = np.zeros((P_pad, K * K), dtype=np.float32)
        if pc:
            corr[:pc] = corr_all[a:b]
        ind = np.zeros((P_pad, G_pad), dtype=np.float32)
        if pc:
            lids = ids[a:b]
            _, lg = np.unique(lids, return_inverse=True)
            ind[np.arange(pc), lg] = 1.0
        ind[pc:, G_pad - 1] = 1.0
        in_maps.append({
            "x": xs,
            "wq64t_bd": wq64t,
            "wkt_bd": wkt,
            "wv_bd": wvb,
            "i119": i119,
            "ind": ind,
            "indT": np.ascontiguousarray(ind.T),
            "corr": corr,
            "bv119": bv119,
        })
    return in_maps, bounds, P_pad, G_pad


def _gather(results, bounds):
    out = np.empty((P_TOTAL, K, 64, 64), dtype=np.float32)
    for ci in range(N_CORES):
        a, b = bounds[ci], bounds[ci + 1]
        pc = b - a
        if pc:
            y = results[ci]["y"][:pc * K].reshape(pc, K, 64, 64)
            out[a:b] = y
    return out


def _run(inputs: dict, trace: bool = False):
    _ensure_path()
    from concourse.bass_utils import run_bass_kernel_spmd

    in_maps, bounds, P_pad, G_pad = _prepare(inputs)
    nc = _get_compiled(P_pad, G_pad)
    res = run_bass_kernel_spmd(nc, in_maps, list(range(N_CORES)), trace=trace)
    return _gather(res.results, bounds), res


def kernel(**inputs) -> np.ndarray:
    out, _ = _run(inputs, trace=False)
    return out



# revision 6
# speedup vs baseline: 1.9991x; 1.9991x over previous
"""Trainium2 Bass kernel for nn_JointRelationModule (self-contained).

Math (per person p, softmax within one imgid group over the person dim):
    q = Wq x ; k = Wk x ; v = Wv x (+b*)        (1x1 conv over K=17 channels)
    S_p = q_p k_p^T / 64                        ([17,17] scores)
    attn = segment-softmax over persons (per imgid group, per (i,j) entry)
    out = relu(attn_p @ v_p + x_p)

Device formulation (per 7-person stack, BDK=119 channels on partitions):
  - G_s = Xt_s^T Xt_s accumulated over 32 hw-chunks of the host-uploaded
    transposed fp16 x (no on-device transposes of the big tensor).
  - masked block-diag G -> M1 = G @ BD(Wq^T/64) -> Z = Wkstack^T @ M1 gives
    scores^T for all 7 persons de-overlapped into a [17, 119] tile; exp on
    ACT writes E in fp16.
  - segment softmax without any gather/scatter DMAs: E^T via a tiny PE
    transpose, segment sums accumulate across stacks via kron(ind, I17)
    indicator matmuls (raggedness is data; program is SPMD-uniform),
    reciprocal on DVE, broadcast back via the transposed indicators.
  - V2 = RepWv^T @ A gives (attn_p Wv) replicated over block-rows; mask to
    block-diag and add I (folds the +x residual into the matmul); then
    out = (AT+I)^T @ x_fp16 per 512-col chunk, relu+bias on ACT/DVE
    alternating, fp16 output assembled per stack and written with one DMA.

Sharding: data-parallel over persons, split at imgid group boundaries
(8 cores), weights replicated. Output returned as f32 (host upcast).
"""

import math
import sys

import numpy as np

K = 17
HW = 4096  # 64*64
P_TOTAL = 512
N_CORES = 8
NORM = 64.0
BD = 7          # persons per stack
BDK = BD * K    # 119
D_CH = 128      # hw chunk for gram contraction
N_DCH = HW // D_CH  # 32
O_CH = 512      # output chunk along hw dim

_cache: dict = {}


def _ensure_path():
    try:
        import concourse.bass  # noqa: F401
    except ImportError:
        for p in ("/opt/trn_rl_repo", "/root/.axon_site/_ro/trn_rl_repo"):
            if p not in sys.path:
                sys.path.insert(0, p)
        import concourse.bass  # noqa: F401


def _build(P_pad: int, n_gh: int):
    """Builds + compiles the per-core SPMD Bass program."""
    _ensure_path()
    import concourse.bacc as bacc
    import concourse.mybir as mybir
    import concourse.tile as tile

    f32 = mybir.dt.float32
    f16 = mybir.dt.float16
    Exp = mybir.ActivationFunctionType.Exp
    Relu = mybir.ActivationFunctionType.Relu
    Add = mybir.AluOpType.add
    Max = mybir.AluOpType.max

    S = P_pad // BD
    assert P_pad % BD == 0 and P_pad <= 128 and n_gh <= 2
    n_och = HW // O_CH  # 8

    nc = bacc.Bacc(
        "TRN2",
        target_bir_lowering=False,
        debug=False,
        enable_asserts=False,
        num_devices=N_CORES,
    )

    xt_d = nc.dram_tensor("xt", [128, S * N_DCH * BDK], f16, kind="ExternalInput")
    xp_d = nc.dram_tensor("xp", [P_pad * K, HW], f16, kind="ExternalInput")
    wqbd_d = nc.dram_tensor("wqbd", [BDK, BDK], f32, kind="ExternalInput")
    wkst_d = nc.dram_tensor("wkst", [BDK, K], f32, kind="ExternalInput")
    wvrep_d = nc.dram_tensor("wvrep", [K, BDK], f16, kind="ExternalInput")
    mask_d = nc.dram_tensor("maskbd", [BDK, BDK], f32, kind="ExternalInput")
    id_d = nc.dram_tensor("idbd", [BDK, BDK], f32, kind="ExternalInput")
    id17_d = nc.dram_tensor("id17", [K, K], f16, kind="ExternalInput")
    corr_d = nc.dram_tensor("corrz", [K, S * BDK], f32, kind="ExternalInput")
    bv_d = nc.dram_tensor("bv17", [K, 1], f16, kind="ExternalInput")
    ind2_d = [nc.dram_tensor(f"ind2_{h}", [BDK, S * BDK], f16,
                             kind="ExternalInput") for h in range(n_gh)]
    ind2t_d = [nc.dram_tensor(f"ind2t_{h}", [BDK, S * BDK], f16,
                              kind="ExternalInput") for h in range(n_gh)]
    y_d = nc.dram_tensor("y", [P_pad * K, HW], f16, kind="ExternalOutput")

    with tile.TileContext(nc) as tc:
        with (
            tc.tile_pool(name="xpool", bufs=1) as xpool,
            tc.tile_pool(name="cpool", bufs=1) as cpool,
            tc.tile_pool(name="wpool", bufs=2) as wpool,
            tc.tile_pool(name="fpool", bufs=1) as fpool,
            tc.tile_pool(name="opool", bufs=2) as opool,
            tc.tile_pool(name="pp", bufs=2, space="PSUM") as pp,
        ):
            # --- replicated constants (scalar/ACT queue) ---
            wqbd_t = cpool.tile([BDK, BDK], f32, name="wqbd_t", tag="wq")
            wkst_t = cpool.tile([BDK, K], f32, name="wkst_t", tag="wk")
            wvrep_t = cpool.tile([K, BDK], f16, name="wvrep_t", tag="wv")
            mask_t = cpool.tile([BDK, BDK], f32, name="mask_t", tag="mask")
            id_t = cpool.tile([BDK, BDK], f32, name="id_t", tag="id")
            id17_t = cpool.tile([K, K], f16, name="id17_t", tag="id17")
            corr_t = cpool.tile([K, S * BDK], f32, name="corr_t", tag="corr")
            bv_t = cpool.tile([K, 1], f16, name="bv_t", tag="bv")
            ind2_t = [cpool.tile([BDK, S * BDK], f16, name=f"ind2_{h}",
                                 tag=f"ind2_{h}") for h in range(n_gh)]
            ind2t_t = [cpool.tile([BDK, S * BDK], f16, name=f"ind2t_{h}",
                                  tag=f"ind2t_{h}") for h in range(n_gh)]
            nc.scalar.dma_start(wqbd_t[:], wqbd_d.ap())
            nc.scalar.dma_start(wkst_t[:], wkst_d.ap())
            nc.scalar.dma_start(wvrep_t[:], wvrep_d.ap())
            nc.scalar.dma_start(mask_t[:], mask_d.ap())
            nc.scalar.dma_start(id_t[:], id_d.ap())
            nc.scalar.dma_start(id17_t[:], id17_d.ap())
            nc.scalar.dma_start(corr_t[:], corr_d.ap())
            nc.scalar.dma_start(bv_t[:], bv_d.ap())
            for h in range(n_gh):
                nc.scalar.dma_start(ind2_t[h][:], ind2_d[h].ap())
                nc.scalar.dma_start(ind2t_t[h][:], ind2t_d[h].ap())

            # --- bulk loads: xt per stack first (gates gram), then xp ---
            xt_sb = xpool.tile([128, S * N_DCH * BDK], f16, name="xt_sb", tag="xt")
            xp_tiles = []
            for s in range(S):
                sl = slice(s * N_DCH * BDK, (s + 1) * N_DCH * BDK)
                nc.sync.dma_start(xt_sb[:, sl], xt_d.ap()[:, sl])
            for s in range(S):
                xps = xpool.tile([BDK, HW], f16, name=f"xp{s}", tag=f"xp{s}")
                nc.sync.dma_start(
                    xps[:], xp_d.ap()[BDK * s:BDK * (s + 1), :]
                )
                xp_tiles.append(xps)

            eall = fpool.tile([K, S * BDK], f16, name="eall", tag="eall")
            a_t = fpool.tile([K, S * BDK], f16, name="a_t", tag="a")
            seg_ps = [pp.tile([BDK, K], f32, name=f"seg{h}", tag=f"seg{h}",
                              bufs=1) for h in range(n_gh)]

            # --- phase A: per stack gram -> chain -> exp -> E^T -> seg acc ---
            for s in range(S):
                zsl = slice(s * BDK, (s + 1) * BDK)
                g_ps = pp.tile([BDK, BDK], f32, name=f"g{s}", tag="g", bufs=2)
                base = s * N_DCH * BDK
                for c in range(N_DCH):
                    op = xt_sb[:, base + c * BDK: base + (c + 1) * BDK]
                    nc.tensor.matmul(
                        g_ps[:], op, op,
                        start=(c == 0), stop=(c == N_DCH - 1),
                    )
                g_sb = wpool.tile([BDK, BDK], f32, name="g_sb", tag="gsb")
                nc.vector.tensor_mul(g_sb[:], g_ps[:], mask_t[:])
                m1_ps = pp.tile([BDK, BDK], f32, name="m1", tag="tiny", bufs=2)
                nc.tensor.matmul(m1_ps[:], g_sb[:], wqbd_t[:], start=True, stop=True)
                m1_sb = wpool.tile([BDK, BDK], f32, name="m1_sb", tag="m1")
                nc.scalar.copy(m1_sb[:], m1_ps[:])
                z_ps = pp.tile([K, BDK], f32, name="z", tag="tiny", bufs=2)
                nc.tensor.matmul(z_ps[:], wkst_t[:], m1_sb[:], start=True, stop=True)
                zc_sb = wpool.tile([K, BDK], f32, name="zc_sb", tag="zc")
                nc.vector.tensor_add(zc_sb[:], z_ps[:], corr_t[:, zsl])
                nc.scalar.activation(eall[:, zsl], zc_sb[:], Exp)
                et_ps = pp.tile([BDK, K], f16, name="et", tag="tiny", bufs=2)
                nc.tensor.transpose(et_ps[:], eall[:, zsl], id17_t[:])
                et_sb = wpool.tile([BDK, K], f16, name="et_sb", tag="et")
                nc.vector.tensor_copy(et_sb[:], et_ps[:])
                for h in range(n_gh):
                    nc.tensor.matmul(
                        seg_ps[h][:], ind2_t[h][:, zsl], et_sb[:],
                        start=(s == 0), stop=(s == S - 1),
                    )

            # --- phase C: reciprocal of group sums; broadcast back per stack ---
            # clamp the reciprocal so empty group slots (seg=0) stay finite in
            # fp16; their zero indicator columns make them exact zeros later.
            inv_sb = []
            for h in range(n_gh):
                segc = fpool.tile([BDK, K], f32, name=f"segc{h}", tag=f"segc{h}")
                nc.vector.tensor_scalar_max(segc[:], seg_ps[h][:], 1e-30)
                invf = fpool.tile([BDK, K], f32, name=f"invf{h}", tag=f"invf{h}")
                nc.vector.reciprocal(invf[:], segc[:])
                inv = fpool.tile([BDK, K], f16, name=f"inv{h}", tag=f"inv{h}")
                nc.vector.tensor_scalar_min(inv[:], invf[:], 60000.0)
                inv_sb.append(inv)

            # --- phase D per stack: invB -> A -> AT(+I) -> out chunks ---
            for s in range(S):
                zsl = slice(s * BDK, (s + 1) * BDK)
                invb_ps = pp.tile([K, BDK], f32, name="invb", tag="tiny", bufs=2)
                for h in range(n_gh):
                    nc.tensor.matmul(
                        invb_ps[:], inv_sb[h][:], ind2t_t[h][:, zsl],
                        start=(h == 0), stop=(h == n_gh - 1),
                    )
                nc.vector.tensor_mul(a_t[:, zsl], eall[:, zsl], invb_ps[:])

                v2_ps = pp.tile([BDK, BDK], f32, name="v2", tag="tiny", bufs=2)
                nc.tensor.matmul(v2_ps[:], wvrep_t[:], a_t[:, zsl],
                                 start=True, stop=True)
                t1_sb = wpool.tile([BDK, BDK], f32, name="t1_sb", tag="t1")
                nc.vector.tensor_mul(t1_sb[:], v2_ps[:], mask_t[:])
                at_sb = wpool.tile([BDK, BDK], f16, name="at_sb", tag="atsb")
                nc.vector.tensor_add(at_sb[:], t1_sb[:], id_t[:])
                av_ps = pp.tile([BDK, 1], f32, name="av", tag="tiny", bufs=2)
                nc.tensor.matmul(av_ps[:], a_t[:, zsl], bv_t[:],
                                 start=True, stop=True)
                av_sb = wpool.tile([BDK, 1], f32, name="av_sb", tag="avsb")
                nc.vector.tensor_copy(av_sb[:], av_ps[:])

                y_sb = opool.tile([BDK, HW], f16, name="y_sb", tag="ysb", bufs=2)
                for oc in range(n_och):
                    osl = slice(O_CH * oc, O_CH * (oc + 1))
                    o_ps = pp.tile([BDK, O_CH], f32, name="o_ps", tag="ops",
                                   bufs=2)
                    nc.tensor.matmul(
                        o_ps[:], at_sb[:], xp_tiles[s][:, osl],
                        start=True, stop=True,
                    )
                    if oc % 2 == 0:
                        nc.scalar.activation(y_sb[:, osl], o_ps[:], Relu,
                                             bias=av_sb[:, 0:1])
                    else:
                        nc.vector.tensor_scalar(
                            y_sb[:, osl], o_ps[:], av_sb[:, 0:1], 0.0,
                            op0=Add, op1=Max,
                        )
                nc.gpsimd.dma_start(
                    y_d.ap()[BDK * s:BDK * (s + 1), :], y_sb[:]
                )

    nc.compile()
    return nc


def _get_compiled(P_pad: int, n_gh: int):
    key = (P_pad, n_gh)
    if key not in _cache:
        _cache[key] = _build(P_pad, n_gh)
    return _cache[key]


def _plan(ids: np.ndarray):
    """Split persons into N_CORES contiguous chunks at imgid boundaries."""
    change = np.flatnonzero(np.diff(ids)) + 1
    allb = np.concatenate([[0], change, [P_TOTAL]]).astype(np.int64)
    bounds = [0]
    for ci in range(1, N_CORES):
        target = P_TOTAL * ci / N_CORES
        cand = allb[allb > bounds[-1]]
        if len(cand) == 0:
            bounds.append(bounds[-1])
        else:
            bounds.append(int(cand[np.argmin(np.abs(cand - target))]))
    bounds.append(P_TOTAL)
    sizes = np.diff(bounds)
    P_max = int(sizes.max())
    P_pad = max(BD, BD * math.ceil(P_max / BD))
    g_max = 0
    for ci in range(N_CORES):
        a, b = bounds[ci], bounds[ci + 1]
        g_max = max(g_max, len(np.unique(ids[a:b])))
    n_gh = math.ceil((g_max + 1) / BD)
    return bounds, P_pad, n_gh


def _prepare(inputs: dict):
    x = np.asarray(inputs["kpt_feat"], dtype=np.float32).reshape(P_TOTAL, K, HW)
    ids = np.asarray(inputs["imgid"]).astype(np.int64)
    Wq = np.asarray(inputs["Wq"], np.float32)
    Wk = np.asarray(inputs["Wk"], np.float32)
    Wv = np.asarray(inputs["Wv"], np.float32)
    bq = np.asarray(inputs["bq"], np.float32)
    bk = np.asarray(inputs["bk"], np.float32)
    bv = np.asarray(inputs["bv"], np.float32)

    bounds, P_pad, n_gh = _plan(ids)
    S = P_pad // BD

    def bd(m):
        out = np.zeros((BDK, BDK), dtype=np.float32)
        for j in range(BD):
            out[K * j:K * (j + 1), K * j:K * (j + 1)] = m
        return out

    wqbd = bd((Wq.T / NORM).astype(np.float32))
    wkst = np.tile(Wk.T.astype(np.float32), (BD, 1))          # [119, 17]
    wvrep = np.tile(Wv.astype(np.float16), (1, BD))           # [17, 119]
    maskbd = bd(np.ones((K, K), np.float32))
    idbd = np.eye(BDK, dtype=np.float32)
    id17 = np.eye(K, dtype=np.float16)
    bv17 = bv.reshape(K, 1).astype(np.float16)
    i17f = np.eye(K, dtype=np.float32)

    have_bias = bool(np.any(bq) or np.any(bk))
    if have_bias:
        xsum = x.sum(axis=2)                    # [P, K]
        qx = xsum @ Wq.T                        # [P, i]
        kx = xsum @ Wk.T                        # [P, m]
        corr_all = (
            bk[None, :, None] * qx[:, None, :]
            + bq[None, None, :] * kx[:, :, None]
            + HW * (bq[None, None, :] * bk[None, :, None])
        ) / NORM                                # [P, m, i]
        corr_all = corr_all.astype(np.float32)
    else:
        corr_all = np.zeros((P_TOTAL, K, K), dtype=np.float32)
    # [m, i] layout to match corr_all; C[i,a] indexed here as [a==m, i]? no:
    # corr_all is [P, m, i] with value for ST[m, i] = scores[i, m]; the shift
    # for scores[i, m] is hw*(Wq_i . Wk_m)/64 -> [m, i] = (Wk Wq^T * hw/64)
    cshift = (HW / NORM) * (Wk @ Wq.T)          # [m, i]

    in_maps = []
    for ci in range(N_CORES):
        a, b = bounds[ci], bounds[ci + 1]
        pc = b - a
        xpad = np.zeros((P_pad, K, HW), dtype=np.float32)
        if pc:
            xpad[:pc] = x[a:b]
        x16 = xpad.astype(np.float16)
        # xt: [128(hw within chunk), S, 32(chunk), 119] from [S,119,32,128]
        xt = np.ascontiguousarray(
            x16.reshape(S, BDK, N_DCH, D_CH).transpose(3, 0, 2, 1)
        ).reshape(128, S * N_DCH * BDK)
        xp = x16.reshape(P_pad * K, HW)

        # corr bias in Z layout, minus the data-independent expected-score
        # shift C[i,a] = hw*(Wq_i . Wk_a)/64 (cancels exactly in the segment
        # softmax, keeps exp() in fp16 range). Pads (x=0) get plain zero.
        czp = np.zeros((P_pad, K, K), np.float32)
        if pc:
            czp[:pc] = corr_all[a:b] - cshift[None, :, :]
        corrz = np.ascontiguousarray(
            czp.transpose(1, 0, 2)).reshape(K, S * BDK)

        # local group index per person; pads -> dummy group g_max_local
        lg = np.full(P_pad, 0, np.int64)
        ng_local = 0
        if pc:
            _, lgc = np.unique(ids[a:b], return_inverse=True)
            lg[:pc] = lgc
            ng_local = int(lgc.max()) + 1
        lg[pc:] = ng_local  # dummy group for padding
        ind_full = np.zeros((P_pad, BD * n_gh), np.float32)
        ind_full[np.arange(P_pad), lg] = 1.0
        ind2, ind2t = [], []
        for h in range(n_gh):
            ind_h = ind_full[:, BD * h:BD * (h + 1)]          # [P_pad, 7]
            arr = np.einsum('sjg,ik->sjigk',
                            ind_h.reshape(S, BD, BD),
                            i17f).reshape(S, BDK, BDK)
            ind2.append(np.ascontiguousarray(
                arr.transpose(1, 0, 2)).reshape(BDK, S * BDK).astype(np.float16))
            ind2t.append(np.ascontiguousarray(
                arr.transpose(2, 0, 1)).reshape(BDK, S * BDK).astype(np.float16))

        im = {
            "xt": xt,
            "xp": xp,
            "wqbd": wqbd,
            "wkst": wkst,
            "wvrep": wvrep,
            "maskbd": maskbd,
            "idbd": idbd,
            "id17": id17,
            "corrz": corrz,
            "bv17": bv17,
        }
        for h in range(n_gh):
            im[f"ind2_{h}"] = ind2[h]
            im[f"ind2t_{h}"] = ind2t[h]
        in_maps.append(im)
    return in_maps, bounds, P_pad, n_gh


def _gather(results, bounds):
    out = np.empty((P_TOTAL, K, 64, 64), dtype=np.float32)
    for ci in range(N_CORES):
        a, b = bounds[ci], bounds[ci + 1]
        pc = b - a
        if pc:
            y = results[ci]["y"][:pc * K].astype(np.float32)
            out[a:b] = y.reshape(pc, K, 64, 64)
    return out


def _run(inputs: dict, trace: bool = False):
    _ensure_path()
    from concourse.bass_utils import run_bass_kernel_spmd

    in_maps, bounds, P_pad, n_gh = _prepare(inputs)
    nc = _get_compiled(P_pad, n_gh)
    res = run_bass_kernel_spmd(nc, in_maps, list(range(N_CORES)), trace=trace)
    return _gather(res.results, bounds), res


def kernel(**inputs) -> np.ndarray:
    out, _ = _run(inputs, trace=False)
    return out


# revision 8
# speedup vs baseline: 2.0079x; 1.0044x over previous
"""Trainium2 Bass kernel for nn_JointRelationModule (self-contained).

Math (per person p, softmax within one imgid group over the person dim):
    q = Wq x ; k = Wk x ; v = Wv x (+b*)        (1x1 conv over K=17 channels)
    S_p = q_p k_p^T / 64                        ([17,17] scores)
    attn = segment-softmax over persons (per imgid group, per (i,j) entry)
    out = relu(attn_p @ v_p + x_p)

Device formulation (per 7-person stack, BDK=119 channels on partitions):
  - G_s = Xt_s^T Xt_s accumulated over 32 hw-chunks of the host-uploaded
    transposed fp16 x (no on-device transposes of the big tensor).
  - masked block-diag G -> M1 = G @ BD(Wq^T/64) -> Z = Wkstack^T @ M1 gives
    scores^T for all 7 persons de-overlapped into a [17, 119] tile; exp on
    ACT writes E in fp16.
  - segment softmax without any gather/scatter DMAs: E^T via a tiny PE
    transpose, segment sums accumulate across stacks via kron(ind, I17)
    indicator matmuls (raggedness is data; program is SPMD-uniform),
    reciprocal on DVE, broadcast back via the transposed indicators.
  - V2 = RepWv^T @ A gives (attn_p Wv) replicated over block-rows; mask to
    block-diag and add I (folds the +x residual into the matmul); then
    out = (AT+I)^T @ x_fp16 per 512-col chunk, relu+bias on ACT/DVE
    alternating, fp16 output assembled per stack and written with one DMA.

Sharding: data-parallel over persons, split at imgid group boundaries
(8 cores), weights replicated. Output returned as f32 (host upcast).
"""

import math
import sys

import numpy as np

K = 17
HW = 4096  # 64*64
P_TOTAL = 512
N_CORES = 8
NORM = 64.0
BD = 7          # persons per stack
BDK = BD * K    # 119
D_CH = 128      # hw chunk for gram contraction
N_DCH = HW // D_CH  # 32
O_CH = 512      # output chunk along hw dim

_cache: dict = {}


def _ensure_path():
    try:
        import concourse.bass  # noqa: F401
    except ImportError:
        for p in ("/opt/trn_rl_repo", "/root/.axon_site/_ro/trn_rl_repo"):
            if p not in sys.path:
                sys.path.insert(0, p)
        import concourse.bass  # noqa: F401


def _build(P_pad: int, n_gh: int):
    """Builds + compiles the per-core SPMD Bass program."""
    _ensure_path()
    import concourse.bacc as bacc
    import concourse.mybir as mybir
    import concourse.tile as tile

    f32 = mybir.dt.float32
    f16 = mybir.dt.float16
    Exp = mybir.ActivationFunctionType.Exp
    Relu = mybir.ActivationFunctionType.Relu
    Add = mybir.AluOpType.add
    Max = mybir.AluOpType.max

    S = P_pad // BD
    assert P_pad % BD == 0 and P_pad <= 128 and n_gh <= 2
    n_och = HW // O_CH  # 8

    nc = bacc.Bacc(
        "TRN2",
        target_bir_lowering=False,
        debug=False,
        enable_asserts=False,
        num_devices=N_CORES,
    )

    xt_d = nc.dram_tensor("xt", [128, S * N_DCH * BDK], f16, kind="ExternalInput")
    xp_d = nc.dram_tensor("xp", [P_pad * K, HW], f16, kind="ExternalInput")
    wqbd_d = nc.dram_tensor("wqbd", [BDK, BDK], f32, kind="ExternalInput")
    wkst_d = nc.dram_tensor("wkst", [BDK, K], f32, kind="ExternalInput")
    wvrep_d = nc.dram_tensor("wvrep", [K, BDK], f16, kind="ExternalInput")
    mask_d = nc.dram_tensor("maskbd", [BDK, BDK], f32, kind="ExternalInput")
    id_d = nc.dram_tensor("idbd", [BDK, BDK], f32, kind="ExternalInput")
    id17_d = nc.dram_tensor("id17", [K, K], f16, kind="ExternalInput")
    corr_d = nc.dram_tensor("corrz", [K, S * BDK], f32, kind="ExternalInput")
    bv_d = nc.dram_tensor("bv17", [K, 1], f16, kind="ExternalInput")
    ind2_d = [nc.dram_tensor(f"ind2_{h}", [BDK, S * BDK], f16,
                             kind="ExternalInput") for h in range(n_gh)]
    ind2t_d = [nc.dram_tensor(f"ind2t_{h}", [BDK, S * BDK], f16,
                              kind="ExternalInput") for h in range(n_gh)]
    y_d = nc.dram_tensor("y", [P_pad * K, HW], f16, kind="ExternalOutput")

    with tile.TileContext(nc) as tc:
        with (
            tc.tile_pool(name="xpool", bufs=1) as xpool,
            tc.tile_pool(name="cpool", bufs=1) as cpool,
            tc.tile_pool(name="wpool", bufs=2) as wpool,
            tc.tile_pool(name="fpool", bufs=1) as fpool,
            tc.tile_pool(name="opool", bufs=2) as opool,
            tc.tile_pool(name="pp", bufs=2, space="PSUM") as pp,
        ):
            # --- replicated constants (scalar/ACT queue) ---
            wqbd_t = cpool.tile([BDK, BDK], f32, name="wqbd_t", tag="wq")
            wkst_t = cpool.tile([BDK, K], f32, name="wkst_t", tag="wk")
            wvrep_t = cpool.tile([K, BDK], f16, name="wvrep_t", tag="wv")
            mask_t = cpool.tile([BDK, BDK], f32, name="mask_t", tag="mask")
            id_t = cpool.tile([BDK, BDK], f32, name="id_t", tag="id")
            id17_t = cpool.tile([K, K], f16, name="id17_t", tag="id17")
            corr_t = cpool.tile([K, S * BDK], f32, name="corr_t", tag="corr")
            bv_t = cpool.tile([K, 1], f16, name="bv_t", tag="bv")
            ind2_t = [cpool.tile([BDK, S * BDK], f16, name=f"ind2_{h}",
                                 tag=f"ind2_{h}") for h in range(n_gh)]
            ind2t_t = [cpool.tile([BDK, S * BDK], f16, name=f"ind2t_{h}",
                                  tag=f"ind2t_{h}") for h in range(n_gh)]
            # consts on scalar queue, chain-critical ones first
            nc.scalar.dma_start(mask_t[:], mask_d.ap())
            nc.scalar.dma_start(wqbd_t[:], wqbd_d.ap())
            nc.scalar.dma_start(wkst_t[:], wkst_d.ap())
            nc.scalar.dma_start(id17_t[:], id17_d.ap())
            nc.scalar.dma_start(corr_t[:], corr_d.ap())
            for h in range(n_gh):
                nc.scalar.dma_start(ind2_t[h][:], ind2_d[h].ap())
            nc.scalar.dma_start(wvrep_t[:], wvrep_d.ap())
            for h in range(n_gh):
                nc.scalar.dma_start(ind2t_t[h][:], ind2t_d[h].ap())
            nc.scalar.dma_start(id_t[:], id_d.ap())
            nc.scalar.dma_start(bv_t[:], bv_d.ap())

            # --- bulk loads: xt gates gram -> spread across all 3 queues ---
            qs = [nc.sync, nc.gpsimd, nc.scalar]
            xt_sb = xpool.tile([128, S * N_DCH * BDK], f16, name="xt_sb", tag="xt")
            xp_tiles = [
                xpool.tile([BDK, HW], f16, name=f"xp{s}", tag=f"xp{s}")
                for s in range(S)
            ]
            for s in range(S):
                sl = slice(s * N_DCH * BDK, (s + 1) * N_DCH * BDK)
                qs[s % 3].dma_start(xt_sb[:, sl], xt_d.ap()[:, sl])
            for s in range(S):
                qs[(s + 2) % 3].dma_start(
                    xp_tiles[s][:], xp_d.ap()[BDK * s:BDK * (s + 1), :]
                )

            eall = fpool.tile([K, S * BDK], f16, name="eall", tag="eall")
            a_t = fpool.tile([K, S * BDK], f16, name="a_t", tag="a")
            seg_ps = [pp.tile([BDK, K], f32, name=f"seg{h}", tag=f"seg{h}",
                              bufs=1) for h in range(n_gh)]

            # --- phase A: per stack gram -> chain -> exp -> E^T -> seg acc ---
            for s in range(S):
                zsl = slice(s * BDK, (s + 1) * BDK)
                g_ps = pp.tile([BDK, BDK], f32, name=f"g{s}", tag="g", bufs=2)
                base = s * N_DCH * BDK
                for c in range(N_DCH):
                    op = xt_sb[:, base + c * BDK: base + (c + 1) * BDK]
                    nc.tensor.matmul(
                        g_ps[:], op, op,
                        start=(c == 0), stop=(c == N_DCH - 1),
                    )
                g_sb = wpool.tile([BDK, BDK], f32, name="g_sb", tag="gsb")
                nc.vector.tensor_mul(g_sb[:], g_ps[:], mask_t[:])
                m1_ps = pp.tile([BDK, BDK], f32, name="m1", tag="tiny", bufs=1)
                nc.tensor.matmul(m1_ps[:], g_sb[:], wqbd_t[:], start=True, stop=True)
                m1_sb = wpool.tile([BDK, BDK], f32, name="m1_sb", tag="m1")
                nc.scalar.copy(m1_sb[:], m1_ps[:])
                z_ps = pp.tile([K, BDK], f32, name="z", tag="tiny", bufs=1)
                nc.tensor.matmul(z_ps[:], wkst_t[:], m1_sb[:], start=True, stop=True)
                zc_sb = wpool.tile([K, BDK], f32, name="zc_sb", tag="zc")
                nc.vector.tensor_add(zc_sb[:], z_ps[:], corr_t[:, zsl])
                nc.scalar.activation(eall[:, zsl], zc_sb[:], Exp)
                et_ps = pp.tile([BDK, K], f16, name="et", tag="ops", bufs=3)
                nc.tensor.transpose(et_ps[:], eall[:, zsl], id17_t[:])
                et_sb = wpool.tile([BDK, K], f16, name="et_sb", tag="et")
                nc.vector.tensor_copy(et_sb[:], et_ps[:])
                for h in range(n_gh):
                    nc.tensor.matmul(
                        seg_ps[h][:], ind2_t[h][:, zsl], et_sb[:],
                        start=(s == 0), stop=(s == S - 1),
                    )

            # --- phase C: reciprocal of group sums; broadcast back per stack ---
            # clamp the reciprocal so empty group slots (seg=0) stay finite in
            # fp16; their zero indicator columns make them exact zeros later.
            inv_sb = []
            for h in range(n_gh):
                segc = fpool.tile([BDK, K], f32, name=f"segc{h}", tag=f"segc{h}")
                nc.vector.tensor_scalar_max(segc[:], seg_ps[h][:], 1e-30)
                invf = fpool.tile([BDK, K], f32, name=f"invf{h}", tag=f"invf{h}")
                nc.vector.reciprocal(invf[:], segc[:])
                inv = fpool.tile([BDK, K], f16, name=f"inv{h}", tag=f"inv{h}")
                nc.vector.tensor_scalar_min(inv[:], invf[:], 60000.0)
                inv_sb.append(inv)

            # --- phase D per stack: invB -> A -> AT(+I) -> out chunks ---
            for s in range(S):
                zsl = slice(s * BDK, (s + 1) * BDK)
                invb_ps = pp.tile([K, BDK], f32, name="invb", tag="tiny", bufs=1)
                for h in range(n_gh):
                    nc.tensor.matmul(
                        invb_ps[:], inv_sb[h][:], ind2t_t[h][:, zsl],
                        start=(h == 0), stop=(h == n_gh - 1),
                    )
                nc.vector.tensor_mul(a_t[:, zsl], eall[:, zsl], invb_ps[:])

                v2_ps = pp.tile([BDK, BDK], f32, name="v2", tag="tiny", bufs=1)
                nc.tensor.matmul(v2_ps[:], wvrep_t[:], a_t[:, zsl],
                                 start=True, stop=True)
                t1_sb = wpool.tile([BDK, BDK], f32, name="t1_sb", tag="t1")
                nc.vector.tensor_mul(t1_sb[:], v2_ps[:], mask_t[:])
                at_sb = wpool.tile([BDK, BDK], f16, name="at_sb", tag="atsb")
                nc.vector.tensor_add(at_sb[:], t1_sb[:], id_t[:])
                av_ps = pp.tile([BDK, 1], f32, name="av", tag="tiny", bufs=1)
                nc.tensor.matmul(av_ps[:], a_t[:, zsl], bv_t[:],
                                 start=True, stop=True)
                av_sb = wpool.tile([BDK, 1], f32, name="av_sb", tag="avsb")
                nc.vector.tensor_copy(av_sb[:], av_ps[:])

                y_sb = opool.tile([BDK, HW], f16, name="y_sb", tag="ysb", bufs=2)
                for oc in range(n_och):
                    osl = slice(O_CH * oc, O_CH * (oc + 1))
                    o_ps = pp.tile([BDK, O_CH], f32, name="o_ps", tag="ops",
                                   bufs=3)
                    nc.tensor.matmul(
                        o_ps[:], at_sb[:], xp_tiles[s][:, osl],
                        start=True, stop=True,
                    )
                    if oc % 2 == 0:
                        nc.scalar.activation(y_sb[:, osl], o_ps[:], Relu,
                                             bias=av_sb[:, 0:1])
                    else:
                        nc.vector.tensor_scalar(
                            y_sb[:, osl], o_ps[:], av_sb[:, 0:1], 0.0,
                            op0=Add, op1=Max,
                        )
                (nc.gpsimd if s % 2 == 0 else nc.sync).dma_start(
                    y_d.ap()[BDK * s:BDK * (s + 1), :], y_sb[:]
                )

    nc.compile()
    return nc


def _get_compiled(P_pad: int, n_gh: int):
    key = (P_pad, n_gh)
    if key not in _cache:
        _cache[key] = _build(P_pad, n_gh)
    return _cache[key]


def _plan(ids: np.ndarray):
    """Split persons into N_CORES contiguous chunks at imgid boundaries."""
    change = np.flatnonzero(np.diff(ids)) + 1
    allb = np.concatenate([[0], change, [P_TOTAL]]).astype(np.int64)
    bounds = [0]
    for ci in range(1, N_CORES):
        target = P_TOTAL * ci / N_CORES
        cand = allb[allb > bounds[-1]]
        if len(cand) == 0:
            bounds.append(bounds[-1])
        else:
            bounds.append(int(cand[np.argmin(np.abs(cand - target))]))
    bounds.append(P_TOTAL)
    sizes = np.diff(bounds)
    P_max = int(sizes.max())
    P_pad = max(BD, BD * math.ceil(P_max / BD))
    g_max = 0
    for ci in range(N_CORES):
        a, b = bounds[ci], bounds[ci + 1]
        g_max = max(g_max, len(np.unique(ids[a:b])))
    n_gh = math.ceil((g_max + 1) / BD)
    return bounds, P_pad, n_gh


def _prepare(inputs: dict):
    x = np.asarray(inputs["kpt_feat"], dtype=np.float32).reshape(P_TOTAL, K, HW)
    ids = np.asarray(inputs["imgid"]).astype(np.int64)
    Wq = np.asarray(inputs["Wq"], np.float32)
    Wk = np.asarray(inputs["Wk"], np.float32)
    Wv = np.asarray(inputs["Wv"], np.float32)
    bq = np.asarray(inputs["bq"], np.float32)
    bk = np.asarray(inputs["bk"], np.float32)
    bv = np.asarray(inputs["bv"], np.float32)

    bounds, P_pad, n_gh = _plan(ids)
    S = P_pad // BD

    def bd(m):
        out = np.zeros((BDK, BDK), dtype=np.float32)
        for j in range(BD):
            out[K * j:K * (j + 1), K * j:K * (j + 1)] = m
        return out

    wqbd = bd((Wq.T / NORM).astype(np.float32))
    wkst = np.tile(Wk.T.astype(np.float32), (BD, 1))          # [119, 17]
    wvrep = np.tile(Wv.astype(np.float16), (1, BD))           # [17, 119]
    maskbd = bd(np.ones((K, K), np.float32))
    idbd = np.eye(BDK, dtype=np.float32)
    id17 = np.eye(K, dtype=np.float16)
    bv17 = bv.reshape(K, 1).astype(np.float16)
    i17f = np.eye(K, dtype=np.float32)

    have_bias = bool(np.any(bq) or np.any(bk))
    if have_bias:
        xsum = x.sum(axis=2)                    # [P, K]
        qx = xsum @ Wq.T                        # [P, i]
        kx = xsum @ Wk.T                        # [P, m]
        corr_all = (
            bk[None, :, None] * qx[:, None, :]
            + bq[None, None, :] * kx[:, :, None]
            + HW * (bq[None, None, :] * bk[None, :, None])
        ) / NORM                                # [P, m, i]
        corr_all = corr_all.astype(np.float32)
    else:
        corr_all = np.zeros((P_TOTAL, K, K), dtype=np.float32)
    # [m, i] layout to match corr_all; C[i,a] indexed here as [a==m, i]? no:
    # corr_all is [P, m, i] with value for ST[m, i] = scores[i, m]; the shift
    # for scores[i, m] is hw*(Wq_i . Wk_m)/64 -> [m, i] = (Wk Wq^T * hw/64)
    cshift = (HW / NORM) * (Wk @ Wq.T)          # [m, i]

    in_maps = []
    for ci in range(N_CORES):
        a, b = bounds[ci], bounds[ci + 1]
        pc = b - a
        xpad = np.zeros((P_pad, K, HW), dtype=np.float32)
        if pc:
            xpad[:pc] = x[a:b]
        x16 = xpad.astype(np.float16)
        # xt: [128(hw within chunk), S, 32(chunk), 119] from [S,119,32,128]
        xt = np.ascontiguousarray(
            x16.reshape(S, BDK, N_DCH, D_CH).transpose(3, 0, 2, 1)
        ).reshape(128, S * N_DCH * BDK)
        xp = x16.reshape(P_pad * K, HW)

        # corr bias in Z layout, minus the data-independent expected-score
        # shift C[i,a] = hw*(Wq_i . Wk_a)/64 (cancels exactly in the segment
        # softmax, keeps exp() in fp16 range). Pads (x=0) get plain zero.
        czp = np.zeros((P_pad, K, K), np.float32)
        if pc:
            czp[:pc] = corr_all[a:b] - cshift[None, :, :]
        corrz = np.ascontiguousarray(
            czp.transpose(1, 0, 2)).reshape(K, S * BDK)

        # local group index per person; pads -> dummy group g_max_local
        lg = np.full(P_pad, 0, np.int64)
        ng_local = 0
        if pc:
            _, lgc = np.unique(ids[a:b], return_inverse=True)
            lg[:pc] = lgc
            ng_local = int(lgc.max()) + 1
        lg[pc:] = ng_local  # dummy group for padding
        ind_full = np.zeros((P_pad, BD * n_gh), np.float32)
        ind_full[np.arange(P_pad), lg] = 1.0
        ind2, ind2t = [], []
        for h in range(n_gh):
            ind_h = ind_full[:, BD * h:BD * (h + 1)]          # [P_pad, 7]
            arr = np.einsum('sjg,ik->sjigk',
                            ind_h.reshape(S, BD, BD),
                            i17f).reshape(S, BDK, BDK)
            ind2.append(np.ascontiguousarray(
                arr.transpose(1, 0, 2)).reshape(BDK, S * BDK).astype(np.float16))
            ind2t.append(np.ascontiguousarray(
                arr.transpose(2, 0, 1)).reshape(BDK, S * BDK).astype(np.float16))

        im = {
            "xt": xt,
            "xp": xp,
            "wqbd": wqbd,
            "wkst": wkst,
            "wvrep": wvrep,
            "maskbd": maskbd,
            "idbd": idbd,
            "id17": id17,
            "corrz": corrz,
            "bv17": bv17,
        }
        for h in range(n_gh):
            im[f"ind2_{h}"] = ind2[h]
            im[f"ind2t_{h}"] = ind2t[h]
        in_maps.append(im)
    return in_maps, bounds, P_pad, n_gh


def _gather(results, bounds):
    out = np.empty((P_TOTAL, K, 64, 64), dtype=np.float32)
    for ci in range(N_CORES):
        a, b = bounds[ci], bounds[ci + 1]
        pc = b - a
        if pc:
            y = results[ci]["y"][:pc * K].astype(np.float32)
            out[a:b] = y.reshape(pc, K, 64, 64)
    return out


def _run(inputs: dict, trace: bool = False):
    _ensure_path()
    from concourse.bass_utils import run_bass_kernel_spmd

    in_maps, bounds, P_pad, n_gh = _prepare(inputs)
    nc = _get_compiled(P_pad, n_gh)
    res = run_bass_kernel_spmd(nc, in_maps, list(range(N_CORES)), trace=trace)
    return _gather(res.results, bounds), res


def kernel(**inputs) -> np.ndarray:
    out, _ = _run(inputs, trace=False)
    return out
